# revision 26
# baseline (speedup 1.0000x reference)
"""Multi-head attention (B=2, N=2048, C=1024, H=16, D=64) on 8 trn2 cores.

Sharding: core c -> (batch b = c//4, head-group g = c%4 covering 4 heads).
Tensor-parallel over heads: Wq/Wk/Wv split column-wise, Wo row-wise; the
4 partial outputs per batch are summed on the host (+ bias).

Device layout (all activations transposed, [C, seq], so every matmul
contracts over partitions with no on-chip transposes):
  QT/KT panels [f, seq]   projection outputs
  VP panel     [seq, f]   (+ones column per head -> softmax denominator)
  S^T  = Kh @ QhT [sk, sq]  two 64-contraction matmuls per sk chunk
  P^T  = exp(S^T * scale)   split between ACT (table exp) and DVE
                            (Schraudolph int16 bit-trick) to widen the
                            softmax-exp bottleneck
  O'^T = [Vh|1]^T @ P^T  [65, sq]  (row 64 = denominator)
  Y^T  = Wo^T @ (O^T/den) [o, seq]  fp16 partials, summed on host

Pipeline: K-proj is split into 4 seq-blocks so S/exp start ~12us in; the
remaining K blocks, V-proj, Q-proj and Y-proj are drip-fed into the PE
stream between attention matmuls. PV trails exp by 2 sk-chunks so the PE
never waits on the exp engines.
"""

import os
import sys

import numpy as np

sys.path.insert(0, "/opt/trn_rl_repo")

import concourse.bacc as bacc
import concourse.bass as bass
import concourse.tile as tile
from concourse import mybir
from concourse.bass_utils import run_bass_kernel_spmd

F32 = mybir.dt.float32
F16 = mybir.dt.float16
I16 = mybir.dt.int16

B = 2
SEQ = 2048
C = 1024
NH = 4          # heads per core
D = 64
FH = NH * D     # 256: feature slice per core
SCALE = D ** -0.5

N_CORES = 8
CCN = C // 128      # 8 contraction chunks
SQN = SEQ // 512    # 4 query chunks
SKN = SEQ // 128    # 16 key chunks

PV_LAG = 2          # PV trails exp by this many sk chunks
DVE_EXP_SKC = frozenset({1, 3, 5, 9, 11, 13})  # skc slots whose exp runs on DVE
# Schraudolph fp16 bit-trick: i16 = round(s*K1 + K2), bits viewed as fp16
EXP_K1 = SCALE * 1.4426950408889634 * 1024.0
EXP_SIGMA = 0.0579
EXP_K2 = 15.0 * 1024.0 - EXP_SIGMA * 1024.0

LAST_RESULTS = None  # stash for test harness introspection


def build_kernel(tc, qT, kT, vT, wq, wk, wv, wo, yT):
    nc = tc.nc

    with (
        tc.tile_pool(name="weights", bufs=1) as wpool,
        tc.tile_pool(name="panels", bufs=1) as panels,
        tc.tile_pool(name="kin", bufs=CCN * 2) as kpool,
        tc.tile_pool(name="qin", bufs=CCN * SQN) as qpool,
        tc.tile_pool(name="vxp", bufs=SKN) as vpool,
        tc.tile_pool(name="ptile", bufs=6) as ppool,
        tc.tile_pool(name="otile", bufs=3) as opool,
        tc.tile_pool(name="ytile", bufs=4) as ypool,
        tc.tile_pool(name="small", bufs=4) as small,
    ):
        # ---- resident weights ----
        wq_sb = wpool.tile([128, CCN, FH], F16, name="wq_sb", tag="wq")
        wk_sb = wpool.tile([128, CCN, FH], F16, name="wk_sb", tag="wk")
        wv_sb = wpool.tile([128, CCN, FH], F16, name="wv_sb", tag="wv")
        wo_sb = wpool.tile([128, 2, C], F16, name="wo_sb", tag="wo")
        # weight / qin / vx DMA triggers go on the (otherwise idle) GpSimd
        # queue: descriptor generation costs ~0.76us per dma_start on the
        # issuing sequencer, and the Sync queue alone throttles the prologue
        nc.gpsimd.dma_start(out=wk_sb, in_=wk[:, :].rearrange("(n p) m -> p n m", p=128))
        nc.gpsimd.dma_start(out=wv_sb, in_=wv[:, :].rearrange("(n p) m -> p n m", p=128))

        # ---- persistent activation panels ----
        qt_sb = panels.tile([128, 2, SEQ], F16, name="qt_sb", tag="qt")
        kt_sb = panels.tile([128, 2, SEQ], F16, name="kt_sb", tag="kt")
        vp_sb = panels.tile([128, SKN, NH, D + 1], F16, name="vp_sb", tag="vp")
        nc.vector.memset(vp_sb[:, :, :, D:D + 1], 1.0)

        # preload the exp activation-table set while DMA streams
        warm_act = small.tile([1, 8], F32, name="warm_act", tag="wact")
        nc.vector.memset(warm_act, 0.0)
        nc.scalar.activation(
            out=warm_act, in_=warm_act,
            func=mybir.ActivationFunctionType.Exp, scale=1.0,
        )

        # ---- input DMAs (sync-queue order == transfer order) ----
        kin = {}

        def dma_kin(half):
            # half 1 triggers from the Scalar queue (idle until the first
            # exp) so its descriptor gen overlaps half 0's on Sync
            eng = nc.sync if half == 0 else nc.scalar
            for cc in range(CCN):
                t = kpool.tile([128, 1024], F16, name="kin", tag="kin")
                eng.dma_start(
                    out=t,
                    in_=kT[cc * 128:(cc + 1) * 128,
                           half * 1024:(half + 1) * 1024],
                )
                kin[(cc, half)] = t

        qin = {}

        def dma_qin(sqc):
            for cc in range(CCN):
                t = qpool.tile([128, 512], F16, name="qin", tag="qin")
                nc.gpsimd.dma_start(
                    out=t,
                    in_=qT[cc * 128:(cc + 1) * 128, sqc * 512:(sqc + 1) * 512],
                )
                qin[(cc, sqc)] = t

        vx = []

        def dma_vx(lo, hi):
            for skc in range(lo, hi):
                t = vpool.tile([128, CCN, 128], F16, name="vx", tag="vx")
                nc.gpsimd.dma_start(
                    out=t,
                    in_=vT[:, skc * 128:(skc + 1) * 128].rearrange(
                        "(n p) m -> p n m", p=128
                    ),
                )
                vx.append(t)

        dma_kin(0)
        nc.gpsimd.dma_start(out=wq_sb, in_=wq[:, :].rearrange("(n p) m -> p n m", p=128))
        dma_qin(0)
        dma_kin(1)
        dma_vx(0, 16)
        dma_qin(1)
        nc.gpsimd.dma_start(out=wo_sb, in_=wo[:, :].rearrange("(n p) m -> p n m", p=128))
        # qin for sqc 2,3 is triggered inside the loop so these descriptor
        # gens don't delay sqc0/1's partition broadcasts on the GpSimd queue

        # all-ones lhsT for the tail's denominator-broadcast matmul
        ones_sb = small.tile([128, 64], F16, name="ones_sb", tag="ones")
        nc.vector.memset(ones_sb, 1.0)

        def k_sc_matmuls(sc, pool, tag):
            """K projection for one 512-col seq block: MMs + panel copies."""
            kacc = {}
            for cc in range(CCN):
                for fc in range(2):
                    if cc == 0:
                        kacc[fc] = pool.tile([128, 512], F32, name="kacc",
                                             tag=tag)
                    nc.tensor.matmul(
                        out=kacc[fc],
                        lhsT=wk_sb[:, cc, fc * 128:(fc + 1) * 128],
                        rhs=kin[(cc, sc // 2)][
                            :, (sc % 2) * 512:(sc % 2 + 1) * 512
                        ],
                        start=(cc == 0),
                        stop=(cc == CCN - 1),
                    )
            for fc in range(2):
                dst = kt_sb[:, fc, sc * 512:(sc + 1) * 512]
                # prologue (sc 0): ScalarE's queue is busy generating DMA
                # descriptors, so keep the PSUM-freeing copies on DVE there
                if fc == 1 and sc > 0:
                    nc.scalar.copy(out=dst, in_=kacc[fc])
                else:
                    nc.vector.tensor_copy(out=dst, in_=kacc[fc])

        # ---- prologue: K block 0 + Q block 0 ----
        with tc.tile_pool(name="ps_proj", bufs=4, space="PSUM") as ps_proj:
            k_sc_matmuls(0, ps_proj, "pacc")
            qacc = {}
            for cc in range(CCN):
                for fc in range(2):
                    if cc == 0:
                        qacc[fc] = ps_proj.tile([128, 512], F32, name="qacc0",
                                                tag="pacc")
                    nc.tensor.matmul(
                        out=qacc[fc],
                        lhsT=wq_sb[:, cc, fc * 128:(fc + 1) * 128],
                        rhs=qin[(cc, 0)][:, :],
                        start=(cc == 0),
                        stop=(cc == CCN - 1),
                    )
            nc.vector.tensor_copy(out=qt_sb[:, 0, 0:512], in_=qacc[0])
            nc.vector.tensor_copy(out=qt_sb[:, 1, 0:512], in_=qacc[1])

        # ---- attention + projections, software-pipelined over sqc ----
        with (
            tc.tile_pool(name="ps_s", bufs=2, space="PSUM") as ps_s,
            tc.tile_pool(name="ps_o", bufs=2, space="PSUM") as ps_o,
            tc.tile_pool(name="ps_misc", bufs=2, space="PSUM") as ps_misc,
        ):
            def v_proj_unit(skc):
                def u():
                    vacc = ps_misc.tile([128, 512], F32, name="vacc", tag="misc")
                    va = vacc[:, 0:FH]
                    for cc in range(CCN):
                        nc.tensor.matmul(
                            out=va,
                            lhsT=vx[skc][:, cc, :],
                            rhs=wv_sb[:, cc, :],
                            start=(cc == 0),
                            stop=(cc == CCN - 1),
                        )
                    copy = (nc.vector.tensor_copy if skc % 2 == 0
                            else nc.scalar.copy)
                    copy(
                        out=vp_sb[:, skc, :, 0:D],
                        in_=va.rearrange("p (h d) -> p h d", h=NH),
                    )
                return u

            def q_proj_units(sqc):
                state = {}

                def make(fc, cc):
                    def u():
                        if cc == 0:
                            state[fc] = ps_misc.tile(
                                [128, 512], F32, name="qacc", tag="misc"
                            )
                        nc.tensor.matmul(
                            out=state[fc],
                            lhsT=wq_sb[:, cc, fc * 128:(fc + 1) * 128],
                            rhs=qin[(cc, sqc)][:, :],
                            start=(cc == 0),
                            stop=(cc == CCN - 1),
                        )
                        if cc == CCN - 1:
                            nc.vector.tensor_copy(
                                out=qt_sb[:, fc, sqc * 512:(sqc + 1) * 512],
                                in_=state[fc],
                            )
                    return u

                return [make(fc, cc) for fc in range(2) for cc in range(CCN)]

            def y_proj_units(ot_tile, sq_slice, tail=False):
                units = []

                def make(oc):
                    state = {}

                    def mm0():
                        state["y"] = ps_misc.tile(
                            [128, 512], F32, name="yacc", tag="misc"
                        )
                        nc.tensor.matmul(
                            out=state["y"],
                            lhsT=wo_sb[:, 0, oc * 128:(oc + 1) * 128],
                            rhs=ot_tile[:, 0, :],
                            start=True,
                            stop=False,
                        )

                    def mm1():
                        nc.tensor.matmul(
                            out=state["y"],
                            lhsT=wo_sb[:, 1, oc * 128:(oc + 1) * 128],
                            rhs=ot_tile[:, 1, :],
                            start=False,
                            stop=True,
                        )
                        y_sb = ypool.tile([128, 512], F16, name="y", tag="y")
                        if tail and oc % 2 == 1:
                            nc.scalar.copy(out=y_sb, in_=state["y"])
                        else:
                            nc.vector.tensor_copy(out=y_sb, in_=state["y"])
                        nc.sync.dma_start(
                            out=yT[oc * 128:(oc + 1) * 128, sq_slice], in_=y_sb
                        )

                    return [mm0, mm1]

                for oc in range(8):
                    units.extend(make(oc))
                return units

            pending = []
            # V panel chunks 0,1 up front so PV(skc=0) never stalls
            v_proj_unit(0)()
            v_proj_unit(1)()
            pending.extend(v_proj_unit(k) for k in range(2, SKN))
            pending.extend(q_proj_units(1))

            for sqc in range(SQN):
                sq = slice(sqc * 512, (sqc + 1) * 512)
                if sqc + 2 < SQN:
                    dma_qin(sqc + 2)
                ot_sb = opool.tile([128, 2, 512], F16, name="ot", tag="ot")
                for hp in range(2):
                    o_ps = [
                        ps_o.tile([D + 1, 512], F32, name="oacc", tag="oacc")
                        for _ in range(2)
                    ]
                    p_tiles = {}
                    # sk chunks are processed two at a time: the PE pays
                    # ~100ns to switch between 64-deep (S) and 128-deep
                    # (PV/proj) contraction, so batch same-depth matmuls
                    for skb in range(0, SKN + PV_LAG, 2):
                        for skc in (skb, skb + 1):
                            if skc >= SKN:
                                continue
                            sk = slice(skc * 128, (skc + 1) * 128)
                            s_ps = ps_s.tile([128, 1024], F32, name="sacc",
                                             tag="sacc")
                            for h2 in range(2):
                                rows = slice(h2 * 64, (h2 + 1) * 64)
                                nc.tensor.matmul(
                                    out=s_ps[:, h2 * 512:(h2 + 1) * 512],
                                    lhsT=kt_sb[rows, hp, sk],
                                    rhs=qt_sb[rows, hp, sq],
                                    start=True,
                                    stop=True,
                                )
                            p_sb = ppool.tile([128, 1024], F16, name="p",
                                              tag="p")
                            if skc in DVE_EXP_SKC:
                                nc.vector.tensor_scalar(
                                    out=p_sb[:, :].bitcast(I16),
                                    in0=s_ps[:, :],
                                    scalar1=float(EXP_K1),
                                    scalar2=float(EXP_K2),
                                    op0=mybir.AluOpType.mult,
                                    op1=mybir.AluOpType.add,
                                )
                            else:
                                nc.scalar.activation(
                                    out=p_sb,
                                    in_=s_ps[:, :],
                                    func=mybir.ActivationFunctionType.Exp,
                                    scale=SCALE,
                                )
                            p_tiles[skc] = p_sb
                        for pk in (skb - PV_LAG, skb - PV_LAG + 1):
                            if not (0 <= pk < SKN):
                                continue
                            p_sb = p_tiles.pop(pk)
                            for h2 in range(2):
                                nc.tensor.matmul(
                                    out=o_ps[h2],
                                    lhsT=vp_sb[:, pk, hp * 2 + h2, :],
                                    rhs=p_sb[:, h2 * 512:(h2 + 1) * 512],
                                    start=(pk == 0),
                                    stop=(pk == SKN - 1),
                                )
                        for _ in range(2):
                            if pending:
                                pending.pop(0)()
                        # stream the remaining K-projection blocks into the
                        # first pipeline pass, just ahead of the S that reads
                        # them
                        if sqc == 0 and hp == 0 and skb in (2, 6, 10):
                            k_sc_matmuls(skb // 4 + 1, ps_misc, "misc")
                    # normalize: rows 0..63 = O^T, row 64 = sum(exp).
                    last_pair = (sqc == SQN - 1 and hp == 1)
                    o_sbs = {}
                    if not last_pair:
                        # drain both PSUM o-banks first: the next hp's first
                        # PV waits on these, and anything queued after the
                        # den0-DMA-gated reciprocal would delay them
                        for h2 in (0, 1):
                            o_sbs[h2] = small.tile([D + 1, 512], F32,
                                                   name="osb", tag="osb")
                            nc.vector.tensor_copy(out=o_sbs[h2], in_=o_ps[h2])
                    for h2 in ((1, 0) if last_pair else (0, 1)):
                        if last_pair:
                            # latency-critical tail: broadcast the
                            # denominator row with a 1-deep matmul on the
                            # now-idle PE instead of DMA + gpsimd
                            o16 = small.tile([D + 1, 512], F16, name="osb16",
                                             tag="osb16")
                            if h2 == 0:
                                nc.scalar.copy(out=o16, in_=o_ps[h2])
                            else:
                                nc.vector.tensor_copy(out=o16, in_=o_ps[h2])
                            den_ps = ps_s.tile([D, 512], F32, name="denps",
                                               tag="sacc")
                            nc.tensor.matmul(
                                out=den_ps,
                                lhsT=ones_sb[D:D + 1, :],
                                rhs=o16[D:D + 1, :],
                                start=True,
                                stop=True,
                                tile_position=(64, 0),
                            )
                            rec_b = small.tile([D, 512], F32, name="recb",
                                               tag="recb")
                            nc.vector.reciprocal_approx_fast(out=rec_b,
                                                             in_=den_ps)
                            if h2 == 0:
                                nc.vector.tensor_mul(
                                    out=ot_sb[0:D, hp, :],
                                    in0=o16[0:D, :],
                                    in1=rec_b,
                                )
                            else:
                                tmp = small.tile([D, 512], F16, name="otmp",
                                                 tag="otmp")
                                nc.vector.tensor_mul(
                                    out=tmp, in0=o16[0:D, :], in1=rec_b
                                )
                                nc.sync.dma_start(out=ot_sb[D:128, hp, :],
                                                  in_=tmp)
                            continue
                        o_sb = o_sbs[h2]
                        den0 = small.tile([1, 512], F32, name="den0",
                                          tag="den0")
                        # row 64 -> partition 0 (DMA can cross partitions)
                        nc.sync.dma_start(out=den0, in_=o_sb[D:D + 1, :])
                        rec = small.tile([1, 512], F32, name="rec", tag="rec")
                        nc.vector.reciprocal_approx_fast(out=rec, in_=den0)
                        rec_b = small.tile([D, 512], F32, name="recb",
                                           tag="recb")
                        nc.gpsimd.partition_broadcast(rec_b, rec)
                        if h2 == 0:
                            nc.vector.tensor_mul(
                                out=ot_sb[0:D, hp, :],
                                in0=o_sb[0:D, :],
                                in1=rec_b,
                            )
                        else:
                            tmp = small.tile([D, 512], F16, name="otmp",
                                             tag="otmp")
                            nc.vector.tensor_mul(
                                out=tmp, in0=o_sb[0:D, :], in1=rec_b
                            )
                            nc.sync.dma_start(out=ot_sb[D:128, hp, :], in_=tmp)

                assert not pending, f"{len(pending)} drip units left at sqc={sqc}"
                pending = y_proj_units(ot_sb, sq, tail=(sqc == SQN - 1))
                if sqc + 2 < SQN:
                    pending.extend(q_proj_units(sqc + 2))
            for step in pending:
                step()


def build_bass():
    nc = bacc.Bacc("TRN2", target_bir_lowering=False, debug=False,
                   enable_asserts=False)
    qT = nc.dram_tensor("qT", [C, SEQ], F16, kind="ExternalInput").ap()
    kT = nc.dram_tensor("kT", [C, SEQ], F16, kind="ExternalInput").ap()
    vT = nc.dram_tensor("vT", [C, SEQ], F16, kind="ExternalInput").ap()
    wq = nc.dram_tensor("wq", [C, FH], F16, kind="ExternalInput").ap()
    wk = nc.dram_tensor("wk", [C, FH], F16, kind="ExternalInput").ap()
    wv = nc.dram_tensor("wv", [C, FH], F16, kind="ExternalInput").ap()
    wo = nc.dram_tensor("wo", [FH, C], F16, kind="ExternalInput").ap()
    yT = nc.dram_tensor("yT", [C, SEQ], F16, kind="ExternalOutput").ap()
    with tile.TileContext(nc) as tc:
        build_kernel(tc, qT, kT, vT, wq, wk, wv, wo, yT)
    nc.compile()
    return nc


_NC = None


def _get_nc():
    global _NC
    if _NC is None:
        _NC = build_bass()
    return _NC


def make_in_maps(q, k, v, Wq, Wk, Wv, Wo):
    f16 = np.float16
    in_maps = []
    for c in range(N_CORES):
        b, g = divmod(c, 4)
        fs = slice(g * FH, (g + 1) * FH)
        in_maps.append(dict(
            qT=np.ascontiguousarray(q[b].T).astype(f16),
            kT=np.ascontiguousarray(k[b].T).astype(f16),
            vT=np.ascontiguousarray(v[b].T).astype(f16),
            wq=np.ascontiguousarray(Wq[:, fs]).astype(f16),
            wk=np.ascontiguousarray(Wk[:, fs]).astype(f16),
            wv=np.ascontiguousarray(Wv[:, fs]).astype(f16),
            wo=np.ascontiguousarray(Wo[fs, :]).astype(f16),
        ))
    return in_maps


def kernel(q, k, v, Wq, Wk, Wv, Wo, bo):
    global LAST_RESULTS
    q = np.asarray(q, dtype=np.float32)
    k = np.asarray(k, dtype=np.float32)
    v = np.asarray(v, dtype=np.float32)
    Wq = np.asarray(Wq, dtype=np.float32)
    Wk = np.asarray(Wk, dtype=np.float32)
    Wv = np.asarray(Wv, dtype=np.float32)
    Wo = np.asarray(Wo, dtype=np.float32)
    bo = np.asarray(bo, dtype=np.float32)

    nc = _get_nc()
    in_maps = make_in_maps(q, k, v, Wq, Wk, Wv, Wo)
    res = run_bass_kernel_spmd(
        nc, in_maps, list(range(N_CORES)),
        trace=bool(os.environ.get("KERNEL_TRACE")),
    )
    LAST_RESULTS = res

    out = np.zeros((B, SEQ, C), dtype=np.float32)
    for c in range(N_CORES):
        out[c // 4] += res.results[c]["yT"].T.astype(np.float32)
    out += bo
    return out.astype(np.float32)


# revision 27
# speedup vs baseline: 1.0523x; 1.0523x over previous
"""Multi-head attention (B=2, N=2048, C=1024, H=16, D=64) on 8 trn2 cores.

Sharding: core c -> (batch b = c//4, head-group g = c%4 covering 4 heads).
Tensor-parallel over heads: Wq/Wk/Wv split column-wise, Wo row-wise; the
4 partial outputs per batch are summed on the host (+ bias).

Device layout (all activations transposed, [C, seq], so every matmul
contracts over partitions with no on-chip transposes):
  QT/KT panels [f, seq]   projection outputs
  VP panel     [seq, f]   (+ones column per head -> softmax denominator)
  S^T  = Kh @ QhT [sk, sq]  two 64-contraction matmuls per sk chunk
  P^T  = exp(S^T * scale)   split between ACT (table exp) and DVE
                            (Schraudolph int16 bit-trick) to widen the
                            softmax-exp bottleneck
  O'^T = [Vh|1]^T @ P^T  [65, sq]  (row 64 = denominator)
  Y^T  = Wo^T @ (O^T/den) [o, seq]  fp16 partials, summed on host

Pipeline: K-proj is split into 4 seq-blocks so S/exp start ~12us in; the
remaining K blocks, V-proj, Q-proj and Y-proj are drip-fed into the PE
stream between attention matmuls. PV trails exp by 2 sk-chunks so the PE
never waits on the exp engines.
"""

import os
import sys

import numpy as np

sys.path.insert(0, "/opt/trn_rl_repo")

import concourse.bacc as bacc
import concourse.bass as bass
import concourse.tile as tile
from concourse import mybir
from concourse.bass_utils import run_bass_kernel_spmd

F32 = mybir.dt.float32
F16 = mybir.dt.float16
I16 = mybir.dt.int16

B = 2
SEQ = 2048
C = 1024
NH = 4          # heads per core
D = 64
FH = NH * D     # 256: feature slice per core
SCALE = D ** -0.5

N_CORES = 8
CCN = C // 128      # 8 contraction chunks
SQN = SEQ // 512    # 4 query chunks
SKN = SEQ // 128    # 16 key chunks

PV_LAG = 2          # PV trails exp by this many sk chunks
DVE_EXP_SKC = frozenset({1, 3, 5, 9, 11, 13})  # skc slots whose exp runs on DVE
# Schraudolph fp16 bit-trick: i16 = round(s*K1 + K2), bits viewed as fp16
EXP_K1 = SCALE * 1.4426950408889634 * 1024.0
EXP_SIGMA = 0.0579
EXP_K2 = 15.0 * 1024.0 - EXP_SIGMA * 1024.0

LAST_RESULTS = None  # stash for test harness introspection


def build_kernel(tc, qT, kT, vT, wq, wk, wv, wo, yT):
    nc = tc.nc

    with (
        tc.tile_pool(name="weights", bufs=1) as wpool,
        tc.tile_pool(name="panels", bufs=1) as panels,
        tc.tile_pool(name="kin", bufs=CCN * 2) as kpool,
        tc.tile_pool(name="qin", bufs=CCN * SQN) as qpool,
        tc.tile_pool(name="vxp", bufs=SKN) as vpool,
        tc.tile_pool(name="ptile", bufs=6) as ppool,
        tc.tile_pool(name="otile", bufs=3) as opool,
        tc.tile_pool(name="ytile", bufs=4) as ypool,
        tc.tile_pool(name="small", bufs=4) as small,
    ):
        # ---- resident weights ----
        wq_sb = wpool.tile([128, CCN, FH], F16, name="wq_sb", tag="wq")
        wk_sb = wpool.tile([128, CCN, FH], F16, name="wk_sb", tag="wk")
        wv_sb = wpool.tile([128, CCN, FH], F16, name="wv_sb", tag="wv")
        wo_sb = wpool.tile([128, 2, C], F16, name="wo_sb", tag="wo")
        # weight / qin / vx DMA triggers go on the (otherwise idle) GpSimd
        # queue: descriptor generation costs ~0.76us per dma_start on the
        # issuing sequencer, and the Sync queue alone throttles the prologue
        nc.gpsimd.dma_start(out=wk_sb, in_=wk[:, :].rearrange("(n p) m -> p n m", p=128))
        nc.gpsimd.dma_start(out=wv_sb, in_=wv[:, :].rearrange("(n p) m -> p n m", p=128))

        # ---- persistent activation panels ----
        qt_sb = panels.tile([128, 2, SEQ], F16, name="qt_sb", tag="qt")
        kt_sb = panels.tile([128, 2, SEQ], F16, name="kt_sb", tag="kt")
        vp_sb = panels.tile([128, SKN, NH, D + 1], F16, name="vp_sb", tag="vp")
        nc.vector.memset(vp_sb[:, :, :, D:D + 1], 1.0)

        # preload the exp activation-table set while DMA streams
        warm_act = small.tile([1, 8], F32, name="warm_act", tag="wact")
        nc.vector.memset(warm_act, 0.0)
        nc.scalar.activation(
            out=warm_act, in_=warm_act,
            func=mybir.ActivationFunctionType.Exp, scale=1.0,
        )

        # ---- input DMAs (sync-queue order == transfer order) ----
        kin = {}

        def dma_kin(half):
            for cc in range(CCN):
                t = kpool.tile([128, 1024], F16, name="kin", tag="kin")
                nc.sync.dma_start(
                    out=t,
                    in_=kT[cc * 128:(cc + 1) * 128,
                           half * 1024:(half + 1) * 1024],
                )
                kin[(cc, half)] = t

        qin = {}

        def dma_qin(sqc):
            for cc in range(CCN):
                t = qpool.tile([128, 512], F16, name="qin", tag="qin")
                nc.gpsimd.dma_start(
                    out=t,
                    in_=qT[cc * 128:(cc + 1) * 128, sqc * 512:(sqc + 1) * 512],
                )
                qin[(cc, sqc)] = t

        vx = []

        def dma_vx(lo, hi):
            for skc in range(lo, hi):
                t = vpool.tile([128, CCN, 128], F16, name="vx", tag="vx")
                nc.gpsimd.dma_start(
                    out=t,
                    in_=vT[:, skc * 128:(skc + 1) * 128].rearrange(
                        "(n p) m -> p n m", p=128
                    ),
                )
                vx.append(t)

        dma_kin(0)
        nc.gpsimd.dma_start(out=wq_sb, in_=wq[:, :].rearrange("(n p) m -> p n m", p=128))
        dma_qin(0)
        dma_kin(1)
        dma_vx(0, 16)
        dma_qin(1)
        nc.gpsimd.dma_start(out=wo_sb, in_=wo[:, :].rearrange("(n p) m -> p n m", p=128))
        # qin for sqc 2,3 is triggered inside the loop so these descriptor
        # gens don't delay sqc0/1's partition broadcasts on the GpSimd queue

        # all-ones lhsT for the tail's denominator-broadcast matmul
        ones_sb = small.tile([128, 64], F16, name="ones_sb", tag="ones")
        nc.vector.memset(ones_sb, 1.0)

        def k_sc_matmuls(sc, pool, tag):
            """K projection for one 512-col seq block: MMs + panel copies."""
            kacc = {}
            for cc in range(CCN):
                for fc in range(2):
                    if cc == 0:
                        kacc[fc] = pool.tile([128, 512], F32, name="kacc",
                                             tag=tag)
                    nc.tensor.matmul(
                        out=kacc[fc],
                        lhsT=wk_sb[:, cc, fc * 128:(fc + 1) * 128],
                        rhs=kin[(cc, sc // 2)][
                            :, (sc % 2) * 512:(sc % 2 + 1) * 512
                        ],
                        start=(cc == 0),
                        stop=(cc == CCN - 1),
                    )
            for fc in range(2):
                dst = kt_sb[:, fc, sc * 512:(sc + 1) * 512]
                # prologue (sc 0): ScalarE's queue is busy generating DMA
                # descriptors, so keep the PSUM-freeing copies on DVE there
                if fc == 1 and sc > 0:
                    nc.scalar.copy(out=dst, in_=kacc[fc])
                else:
                    nc.vector.tensor_copy(out=dst, in_=kacc[fc])

        # ---- prologue: K block 0 + Q block 0 ----
        with tc.tile_pool(name="ps_proj", bufs=4, space="PSUM") as ps_proj:
            k_sc_matmuls(0, ps_proj, "pacc")
            qacc = {}
            for cc in range(CCN):
                for fc in range(2):
                    if cc == 0:
                        qacc[fc] = ps_proj.tile([128, 512], F32, name="qacc0",
                                                tag="pacc")
                    nc.tensor.matmul(
                        out=qacc[fc],
                        lhsT=wq_sb[:, cc, fc * 128:(fc + 1) * 128],
                        rhs=qin[(cc, 0)][:, :],
                        start=(cc == 0),
                        stop=(cc == CCN - 1),
                    )
            nc.vector.tensor_copy(out=qt_sb[:, 0, 0:512], in_=qacc[0])
            nc.vector.tensor_copy(out=qt_sb[:, 1, 0:512], in_=qacc[1])

        # ---- attention + projections, software-pipelined over sqc ----
        with (
            tc.tile_pool(name="ps_s", bufs=2, space="PSUM") as ps_s,
            tc.tile_pool(name="ps_o", bufs=2, space="PSUM") as ps_o,
            tc.tile_pool(name="ps_misc", bufs=2, space="PSUM") as ps_misc,
        ):
            def v_proj_unit(skc):
                def u():
                    vacc = ps_misc.tile([128, 512], F32, name="vacc", tag="misc")
                    va = vacc[:, 0:FH]
                    for cc in range(CCN):
                        nc.tensor.matmul(
                            out=va,
                            lhsT=vx[skc][:, cc, :],
                            rhs=wv_sb[:, cc, :],
                            start=(cc == 0),
                            stop=(cc == CCN - 1),
                        )
                    copy = (nc.vector.tensor_copy if skc % 2 == 0
                            else nc.scalar.copy)
                    copy(
                        out=vp_sb[:, skc, :, 0:D],
                        in_=va.rearrange("p (h d) -> p h d", h=NH),
                    )
                return u

            def q_proj_units(sqc):
                state = {}

                def make(fc, cc):
                    def u():
                        if cc == 0:
                            state[fc] = ps_misc.tile(
                                [128, 512], F32, name="qacc", tag="misc"
                            )
                        nc.tensor.matmul(
                            out=state[fc],
                            lhsT=wq_sb[:, cc, fc * 128:(fc + 1) * 128],
                            rhs=qin[(cc, sqc)][:, :],
                            start=(cc == 0),
                            stop=(cc == CCN - 1),
                        )
                        if cc == CCN - 1:
                            nc.vector.tensor_copy(
                                out=qt_sb[:, fc, sqc * 512:(sqc + 1) * 512],
                                in_=state[fc],
                            )
                    return u

                return [make(fc, cc) for fc in range(2) for cc in range(CCN)]

            def y_proj_units(ot_tile, sq_slice, tail=False):
                units = []

                def make(oc):
                    state = {}

                    def mm0():
                        state["y"] = ps_misc.tile(
                            [128, 512], F32, name="yacc", tag="misc"
                        )
                        nc.tensor.matmul(
                            out=state["y"],
                            lhsT=wo_sb[:, 0, oc * 128:(oc + 1) * 128],
                            rhs=ot_tile[:, 0, :],
                            start=True,
                            stop=False,
                        )

                    def mm1():
                        nc.tensor.matmul(
                            out=state["y"],
                            lhsT=wo_sb[:, 1, oc * 128:(oc + 1) * 128],
                            rhs=ot_tile[:, 1, :],
                            start=False,
                            stop=True,
                        )
                        y_sb = ypool.tile([128, 512], F16, name="y", tag="y")
                        if tail and oc % 2 == 1:
                            nc.scalar.copy(out=y_sb, in_=state["y"])
                        else:
                            nc.vector.tensor_copy(out=y_sb, in_=state["y"])
                        nc.sync.dma_start(
                            out=yT[oc * 128:(oc + 1) * 128, sq_slice], in_=y_sb
                        )

                    return [mm0, mm1]

                for oc in range(8):
                    units.extend(make(oc))
                return units

            pending = []
            # V panel chunks 0,1 up front so PV(skc=0) never stalls
            v_proj_unit(0)()
            v_proj_unit(1)()
            pending.extend(v_proj_unit(k) for k in range(2, SKN))
            pending.extend(q_proj_units(1))

            for sqc in range(SQN):
                sq = slice(sqc * 512, (sqc + 1) * 512)
                if sqc + 2 < SQN:
                    dma_qin(sqc + 2)
                ot_sb = opool.tile([128, 2, 512], F16, name="ot", tag="ot")
                for hp in range(2):
                    o_ps = [
                        ps_o.tile([D + 1, 512], F32, name="oacc", tag="oacc")
                        for _ in range(2)
                    ]
                    p_tiles = {}
                    # sk chunks are processed two at a time: the PE pays
                    # ~100ns to switch between 64-deep (S) and 128-deep
                    # (PV/proj) contraction, so batch same-depth matmuls
                    for skb in range(0, SKN + PV_LAG, 2):
                        for skc in (skb, skb + 1):
                            if skc >= SKN:
                                continue
                            sk = slice(skc * 128, (skc + 1) * 128)
                            s_ps = ps_s.tile([128, 1024], F32, name="sacc",
                                             tag="sacc")
                            for h2 in range(2):
                                rows = slice(h2 * 64, (h2 + 1) * 64)
                                nc.tensor.matmul(
                                    out=s_ps[:, h2 * 512:(h2 + 1) * 512],
                                    lhsT=kt_sb[rows, hp, sk],
                                    rhs=qt_sb[rows, hp, sq],
                                    start=True,
                                    stop=True,
                                )
                            p_sb = ppool.tile([128, 1024], F16, name="p",
                                              tag="p")
                            if skc in DVE_EXP_SKC:
                                nc.vector.tensor_scalar(
                                    out=p_sb[:, :].bitcast(I16),
                                    in0=s_ps[:, :],
                                    scalar1=float(EXP_K1),
                                    scalar2=float(EXP_K2),
                                    op0=mybir.AluOpType.mult,
                                    op1=mybir.AluOpType.add,
                                )
                            else:
                                nc.scalar.activation(
                                    out=p_sb,
                                    in_=s_ps[:, :],
                                    func=mybir.ActivationFunctionType.Exp,
                                    scale=SCALE,
                                )
                            p_tiles[skc] = p_sb
                        for pk in (skb - PV_LAG, skb - PV_LAG + 1):
                            if not (0 <= pk < SKN):
                                continue
                            p_sb = p_tiles.pop(pk)
                            for h2 in range(2):
                                nc.tensor.matmul(
                                    out=o_ps[h2],
                                    lhsT=vp_sb[:, pk, hp * 2 + h2, :],
                                    rhs=p_sb[:, h2 * 512:(h2 + 1) * 512],
                                    start=(pk == 0),
                                    stop=(pk == SKN - 1),
                                )
                        for _ in range(2):
                            if pending:
                                pending.pop(0)()
                        # stream the remaining K-projection blocks into the
                        # first pipeline pass, just ahead of the S that reads
                        # them
                        if sqc == 0 and hp == 0 and skb in (2, 6, 10):
                            k_sc_matmuls(skb // 4 + 1, ps_misc, "misc")
                    # normalize: rows 0..63 = O^T, row 64 = sum(exp).
                    last_pair = (sqc == SQN - 1 and hp == 1)
                    o_sbs = {}
                    if not last_pair:
                        # drain both PSUM o-banks first: the next hp's first
                        # PV waits on these, and anything queued after the
                        # den0-DMA-gated reciprocal would delay them
                        for h2 in (0, 1):
                            o_sbs[h2] = small.tile([D + 1, 512], F32,
                                                   name="osb", tag="osb")
                            nc.vector.tensor_copy(out=o_sbs[h2], in_=o_ps[h2])
                    for h2 in ((1, 0) if last_pair else (0, 1)):
                        if last_pair:
                            # latency-critical tail: broadcast the
                            # denominator row with a 1-deep matmul on the
                            # now-idle PE instead of DMA + gpsimd
                            o16 = small.tile([D + 1, 512], F16, name="osb16",
                                             tag="osb16")
                            if h2 == 0:
                                nc.scalar.copy(out=o16, in_=o_ps[h2])
                            else:
                                nc.vector.tensor_copy(out=o16, in_=o_ps[h2])
                            den_ps = ps_s.tile([D, 512], F32, name="denps",
                                               tag="sacc")
                            nc.tensor.matmul(
                                out=den_ps,
                                lhsT=ones_sb[D:D + 1, :],
                                rhs=o16[D:D + 1, :],
                                start=True,
                                stop=True,
                                tile_position=(64, 0),
                            )
                            rec_b = small.tile([D, 512], F32, name="recb",
                                               tag="recb")
                            nc.vector.reciprocal_approx_fast(out=rec_b,
                                                             in_=den_ps)
                            if h2 == 0:
                                nc.vector.tensor_mul(
                                    out=ot_sb[0:D, hp, :],
                                    in0=o16[0:D, :],
                                    in1=rec_b,
                                )
                            else:
                                tmp = small.tile([D, 512], F16, name="otmp",
                                                 tag="otmp")
                                nc.vector.tensor_mul(
                                    out=tmp, in0=o16[0:D, :], in1=rec_b
                                )
                                nc.sync.dma_start(out=ot_sb[D:128, hp, :],
                                                  in_=tmp)
                            continue
                        o_sb = o_sbs[h2]
                        den0 = small.tile([1, 512], F32, name="den0",
                                          tag="den0")
                        # row 64 -> partition 0 (DMA can cross partitions)
                        nc.sync.dma_start(out=den0, in_=o_sb[D:D + 1, :])
                        rec = small.tile([1, 512], F32, name="rec", tag="rec")
                        nc.vector.reciprocal_approx_fast(out=rec, in_=den0)
                        rec_b = small.tile([D, 512], F32, name="recb",
                                           tag="recb")
                        nc.gpsimd.partition_broadcast(rec_b, rec)
                        if h2 == 0:
                            nc.vector.tensor_mul(
                                out=ot_sb[0:D, hp, :],
                                in0=o_sb[0:D, :],
                                in1=rec_b,
                            )
                        else:
                            tmp = small.tile([D, 512], F16, name="otmp",
                                             tag="otmp")
                            nc.vector.tensor_mul(
                                out=tmp, in0=o_sb[0:D, :], in1=rec_b
                            )
                            nc.sync.dma_start(out=ot_sb[D:128, hp, :], in_=tmp)

                assert not pending, f"{len(pending)} drip units left at sqc={sqc}"
                pending = y_proj_units(ot_sb, sq, tail=(sqc == SQN - 1))
                if sqc + 2 < SQN:
                    pending.extend(q_proj_units(sqc + 2))
            for step in pending:
                step()


def build_bass():
    nc = bacc.Bacc("TRN2", target_bir_lowering=False, debug=False,
                   enable_asserts=False)
    qT = nc.dram_tensor("qT", [C, SEQ], F16, kind="ExternalInput").ap()
    kT = nc.dram_tensor("kT", [C, SEQ], F16, kind="ExternalInput").ap()
    vT = nc.dram_tensor("vT", [C, SEQ], F16, kind="ExternalInput").ap()
    wq = nc.dram_tensor("wq", [C, FH], F16, kind="ExternalInput").ap()
    wk = nc.dram_tensor("wk", [C, FH], F16, kind="ExternalInput").ap()
    wv = nc.dram_tensor("wv", [C, FH], F16, kind="ExternalInput").ap()
    wo = nc.dram_tensor("wo", [FH, C], F16, kind="ExternalInput").ap()
    yT = nc.dram_tensor("yT", [C, SEQ], F16, kind="ExternalOutput").ap()
    with tile.TileContext(nc) as tc:
        build_kernel(tc, qT, kT, vT, wq, wk, wv, wo, yT)
    nc.compile()
    return nc


_NC = None


def _get_nc():
    global _NC
    if _NC is None:
        _NC = build_bass()
    return _NC


def make_in_maps(q, k, v, Wq, Wk, Wv, Wo):
    f16 = np.float16
    in_maps = []
    for c in range(N_CORES):
        b, g = divmod(c, 4)
        fs = slice(g * FH, (g + 1) * FH)
        in_maps.append(dict(
            qT=np.ascontiguousarray(q[b].T).astype(f16),
            kT=np.ascontiguousarray(k[b].T).astype(f16),
            vT=np.ascontiguousarray(v[b].T).astype(f16),
            wq=np.ascontiguousarray(Wq[:, fs]).astype(f16),
            wk=np.ascontiguousarray(Wk[:, fs]).astype(f16),
            wv=np.ascontiguousarray(Wv[:, fs]).astype(f16),
            wo=np.ascontiguousarray(Wo[fs, :]).astype(f16),
        ))
    return in_maps


def kernel(q, k, v, Wq, Wk, Wv, Wo, bo):
    global LAST_RESULTS
    q = np.asarray(q, dtype=np.float32)
    k = np.asarray(k, dtype=np.float32)
    v = np.asarray(v, dtype=np.float32)
    Wq = np.asarray(Wq, dtype=np.float32)
    Wk = np.asarray(Wk, dtype=np.float32)
    Wv = np.asarray(Wv, dtype=np.float32)
    Wo = np.asarray(Wo, dtype=np.float32)
    bo = np.asarray(bo, dtype=np.float32)

    nc = _get_nc()
    in_maps = make_in_maps(q, k, v, Wq, Wk, Wv, Wo)
    res = run_bass_kernel_spmd(
        nc, in_maps, list(range(N_CORES)),
        trace=bool(os.environ.get("KERNEL_TRACE")),
    )
    LAST_RESULTS = res

    out = np.zeros((B, SEQ, C), dtype=np.float32)
    for c in range(N_CORES):
        out[c // 4] += res.results[c]["yT"].T.astype(np.float32)
    out += bo
    return out.astype(np.float32)
